# revision 1
# baseline (speedup 1.0000x reference)
"""Trainium2 Bass kernel for nn_FAttention1d (attention with softmax over the
QUERY axis).

Reference computation (B=2, H=16, S=2048, D=64, fp32):
    att[b,h,q,k] = sum_d qry[b,h,q,d] * key[b,h,k,d]
    att += reg * I_S                      (diagonal in (q,k))
    att = softmax(att, axis=q)            (normalize over the QUERY axis)
    out[b,h,q,v] = sum_k att[b,h,q,k] * val[b,h,k,v]

Sharding: the 32 (b,h) pairs are split 4-per-core across 8 NeuronCores; the
whole S=2048 attention chain is local to a core.

Device-side layout: compute S^T = K @ Q^T with k on the partition axis, so the
q-axis softmax is a free-axis reduction (fused into the exp pass via the ACT
accumulator), and exp(S^T) tiles feed the A^T V matmul directly as the moving
operand:
    out^T[v,q] = sum_k (val[k,v] / r[k])_stationary @ exp(S^T)[k,q]
with r[k] = sum_q exp(S^T[k,q]) folded into the val rows (S*D elements instead
of S*S).

Host-side prep (outside the measured device kernel): per-core shard slicing,
q/k transposed to [D, S], val swizzled so each [128, 64] k-tile is contiguous,
reg materialized as a [128,128] diagonal tile, and the final out^T -> out
transpose.
"""

import numpy as np
from collections import deque
from contextlib import ExitStack

import concourse.bass as bass
import concourse.mybir as mybir
import concourse.tile as tile
from concourse import bacc
from concourse.bass_utils import run_bass_kernel_spmd

B, H, S, D = 2, 16, 2048, 64
N_CORES = 8
BH = B * H                     # 32
BH_PER_CORE = BH // N_CORES    # 4
NT = S // 128                  # 16 k-tiles of 128
NG = NT // 4                   # 4 groups of 4 k-tiles
F32 = mybir.dt.float32
F32R = mybir.dt.float32r
F16 = mybir.dt.float16
BF16 = mybir.dt.bfloat16


def _build_kernel(nc, tc, ctx, qt, kt, vs, rg, ot):
    const_pool = ctx.enter_context(tc.tile_pool(name="const", bufs=1))
    q_pool = ctx.enter_context(tc.tile_pool(name="q", bufs=2))
    k_pool = ctx.enter_context(tc.tile_pool(name="k", bufs=2))
    v_pool = ctx.enter_context(tc.tile_pool(name="v", bufs=1))
    e_pool = ctx.enter_context(tc.tile_pool(name="e", bufs=8))
    r_pool = ctx.enter_context(tc.tile_pool(name="r", bufs=2))
    vsc_pool = ctx.enter_context(tc.tile_pool(name="vsc", bufs=6))
    osb_pool = ctx.enter_context(tc.tile_pool(name="osb", bufs=2))
    stg_pool = ctx.enter_context(tc.tile_pool(name="stg", bufs=4))
    st_pool = ctx.enter_context(tc.tile_pool(name="st", bufs=2, space="PSUM"))
    o_pool = ctx.enter_context(tc.tile_pool(name="o", bufs=1, space="PSUM"))

    rg_eye = const_pool.tile([128, 128], F32)
    nc.gpsimd.dma_start(rg_eye[:], rg[:])

    AB = (0, 1)
    for p in range(BH_PER_CORE // 2):
        bh = (2 * p, 2 * p + 1)
        q2 = q_pool.tile([128, S], F16, tag="q2", name="q2")
        k2 = k_pool.tile([128, S], F16, tag="k2", name="k2")
        nc.sync.dma_start(q2[:, 0:512], qt[p][:, 0:512])
        nc.gpsimd.dma_start(k2[:, 0:128], kt[p][:, 0:128])
        nc.sync.dma_start(q2[:, 512:], qt[p][:, 512:])
        nc.gpsimd.dma_start(k2[:, 128:], kt[p][:, 128:])
        v_sb = [None, None]
        for s in AB:
            v_sb[s] = v_pool.tile([128, NT * 64], F32, tag=f"v{s}", name=f"v_sb{s}")
            nc.gpsimd.dma_start(v_sb[s][:], vs[bh[s]])

        # out^T for the pair: partitions 0-63 = bh A, 64-127 = bh B
        o_ps = o_pool.tile([128, S], F32)
        r_all = [r_pool.tile([128, 2, NT], F32, tag=f"rall{s}", name=f"r_all{s}") for s in AB]
        r_sum = [r_pool.tile([128, NT], F32, tag=f"rsum{s}", name=f"r_sum{s}") for s in AB]
        r_inv = [r_pool.tile([128, NT], F32, tag=f"rinv{s}", name=f"r_inv{s}") for s in AB]
        e_tiles = [[None] * NT, [None] * NT]
        vsc_tiles = [[None] * NT, [None] * NT]
        pending = deque()

        def queue_av_group(g):
            # enqueue vsc scales + col-packed AV matmuls for group g; they are
            # drained a few at a time between QK chunks so the PE never stops
            # feeding ACT with fresh st tiles
            for m in range(4 * g, 4 * g + 4):
                pending.append(("vsc", m))
                for h in range(4):
                    pending.append(("av", m, h))

        def drain_pending(k=3):
            for _ in range(k):
                if not pending:
                    return
                item = pending.popleft()
                if item[0] == "vsc":
                    m = item[1]
                    for s in AB:
                        vsc = vsc_pool.tile([128, 64], BF16, tag=f"vsc{s}",
                                            name=f"vsc{s}")
                        vsc_tiles[s][m] = vsc
                        nc.vector.tensor_scalar_mul(
                            vsc[:], v_sb[s][:, m * 64:(m + 1) * 64],
                            r_inv[s][:, m:m + 1]
                        )
                else:
                    _, m, h = item
                    ch = slice(h * 512, (h + 1) * 512)
                    for s in AB:
                        # col-packed pair: bh A -> out partitions 0-63,
                        # bh B -> 64-127 (tile_position auto (0, 64*s))
                        nc.tensor.matmul(
                            o_ps[64 * s:64 * s + 64, ch],
                            lhsT=vsc_tiles[s][m][:],
                            rhs=e_tiles[s][m][:, ch],
                            start=(m == 0),
                            stop=(m == NT - 1),
                            skip_group_check=True,
                        )

        for n in range(NT):
            for s in AB:
                e_tiles[s][n] = e_pool.tile([128, S], BF16, tag=f"e{s}", name=f"e{s}_{n}")
            stage = stg_pool.tile([128, S], F32, tag="stg", name="stage")
            stage0 = (stg_pool.tile([128, S], F32, tag="stg0", name="stage0", bufs=2)
                      if n % 2 == 1 else None)
            for h in range(2):
                for s in AB:
                    st = st_pool.tile([128, 1024], F32)
                    for j in range(2):
                        q0 = h * 1024 + j * 512
                        nc.tensor.matmul(
                            st[:, j * 512:(j + 1) * 512],
                            lhsT=k2[64 * s:64 * s + 64, n * 128:(n + 1) * 128],
                            rhs=q2[64 * s:64 * s + 64, q0:q0 + 512],
                            start=True,
                            stop=True,
                        )
                    # diagonal of S^T for k-tile n sits at q-columns
                    # n*128..n*128+127; add reg*I if in this half
                    if n // 8 == h:
                        c = (n % 8) * 128
                        nc.vector.tensor_add(
                            st[:, c:c + 128], st[:, c:c + 128], rg_eye[:]
                        )
                    if s == 0 and n % 2 == 0:
                        # bh A, even k-tiles: exp straight from PSUM per half
                        nc.scalar.activation(
                            e_tiles[0][n][:, h * 1024:(h + 1) * 1024],
                            st[:],
                            mybir.ActivationFunctionType.Exp,
                            accum_out=r_all[0][:, h:h + 1, n:n + 1],
                        )
                    elif s == 0:
                        nc.vector.tensor_copy(
                            stage0[:, h * 1024:(h + 1) * 1024], st[:])
                        if h == 1:
                            nc.scalar.activation(
                                e_tiles[0][n][:],
                                stage0[:],
                                mybir.ActivationFunctionType.Exp,
                                accum_out=r_sum[0][:, n:n + 1],
                            )
                    else:
                        # bh B: stage to SBUF (cheap DVE copy) so ACT can do
                        # one [128, 2048] exp -- fewer per-op overheads
                        nc.vector.tensor_copy(
                            stage[:, h * 1024:(h + 1) * 1024], st[:])
                        if h == 1:
                            nc.scalar.activation(
                                e_tiles[1][n][:],
                                stage[:],
                                mybir.ActivationFunctionType.Exp,
                                accum_out=r_sum[1][:, n:n + 1],
                            )
                drain_pending(3)
            if n % 4 == 3:
                g = n // 4
                gs = slice(4 * g, 4 * g + 4)
                for nn in range(4 * g, 4 * g + 4, 2):
                    nc.vector.tensor_add(
                        r_sum[0][:, nn:nn + 1], r_all[0][:, 0, nn:nn + 1],
                        r_all[0][:, 1, nn:nn + 1]
                    )
                for s in AB:
                    nc.vector.reciprocal_approx_fast(
                        r_inv[s][:, gs], r_sum[s][:, gs])
                queue_av_group(g)
        while pending:
            drain_pending(4)

        out_sb = osb_pool.tile([128, S], F32)
        for h in range(4):
            ch = slice(h * 512, (h + 1) * 512)
            nc.vector.tensor_copy(out_sb[:, ch], o_ps[:, ch])
        for s in AB:
            nc.gpsimd.dma_start(ot[bh[s]], out_sb[64 * s:64 * s + 64, :])


_NC_CACHE = {}


def build_nc(repeats=1):
    key = repeats
    if key in _NC_CACHE:
        return _NC_CACHE[key]
    nc = bacc.Bacc("TRN2", target_bir_lowering=False, debug=False)
    qt = nc.dram_tensor("qt", [BH_PER_CORE // 2, 2 * D, S], F16, kind="ExternalInput").ap()
    kt = nc.dram_tensor("kt", [BH_PER_CORE // 2, 2 * D, S], F16, kind="ExternalInput").ap()
    vs = nc.dram_tensor("vs", [BH_PER_CORE, 128, NT * 64], F32, kind="ExternalInput").ap()
    rg = nc.dram_tensor("rg", [128, 128], F32, kind="ExternalInput").ap()
    ot = nc.dram_tensor("ot", [BH_PER_CORE, D, S], F32, kind="ExternalOutput").ap()
    with tile.TileContext(nc) as tc, ExitStack() as ctx:
        if repeats == 1:
            _build_kernel(nc, tc, ctx, qt, kt, vs, rg, ot)
        else:
            # benchmarking mode: repeat the whole kernel body in an on-device
            # loop so per-iteration time can be extracted from wall clock
            with tc.For_i(0, repeats, 1,
                          hint_engines=(mybir.EngineType.PE,
                                        mybir.EngineType.Activation,
                                        mybir.EngineType.DVE)):
                _build_kernel(nc, tc, ctx, qt, kt, vs, rg, ot)
    nc.compile()
    _NC_CACHE[key] = nc
    return nc


def _prep_inputs(qry, key, val, reg):
    """Host-side shard + layout prep. Returns per-core input maps."""
    q = np.ascontiguousarray(np.asarray(qry, dtype=np.float32)).reshape(BH, S, D)
    k = np.ascontiguousarray(np.asarray(key, dtype=np.float32)).reshape(BH, S, D)
    v = np.ascontiguousarray(np.asarray(val, dtype=np.float32)).reshape(BH, S, D)
    rg = (np.eye(128, dtype=np.float32) * np.float32(np.asarray(reg)))

    in_maps = []
    for c in range(N_CORES):
        sl = slice(c * BH_PER_CORE, (c + 1) * BH_PER_CORE)
        qt = np.ascontiguousarray(
            q[sl].transpose(0, 2, 1).reshape(BH_PER_CORE // 2, 2 * D, S)
        ).astype(np.float16)                                          # [2, 128, S]
        kt = np.ascontiguousarray(
            k[sl].transpose(0, 2, 1).reshape(BH_PER_CORE // 2, 2 * D, S)
        ).astype(np.float16)                                          # [2, 128, S]
        vv = v[sl].reshape(BH_PER_CORE, NT, 128, D)
        vs = np.ascontiguousarray(vv.transpose(0, 2, 1, 3)).reshape(
            BH_PER_CORE, 128, NT * D)                                 # [4, 128, 1024]
        in_maps.append({"qt": qt, "kt": kt, "vs": vs, "rg": rg})
    return in_maps


def kernel(qry, key, val, reg):
    nc = build_nc()
    in_maps = _prep_inputs(qry, key, val, reg)
    res = run_bass_kernel_spmd(nc, in_maps, list(range(N_CORES)))
    out = np.empty((BH, S, D), dtype=np.float32)
    for c in range(N_CORES):
        ot = res.results[c]["ot"]                                    # [4, 64, S]
        for i in range(BH_PER_CORE):
            out[c * BH_PER_CORE + i] = ot[i].T
    return out.reshape(B, H, S, D)

